# revision 16
# baseline (speedup 1.0000x reference)
"""Trainium2 Bass kernel for nn_Attention_68006512164916.

EVA-style vision attention block: qkv proj -> 2D rope (interleaved pairs)
-> SDPA (16 heads, d=64, seq 256) -> out proj. B=64, N=256, C=1024, fp32 I/O.

Strategy: data-parallel over batch across 8 NeuronCores (8 batches/core,
no collectives). Per core, everything is computed in bf16 on the
TensorEngine with fp32 PSUM accumulation:

  - host: x is transposed/cast to xT [C, B_loc*N] bf16 so the QKV matmul
    needs no on-device transpose of x; qkv_w rows for q/k are permuted
    (per-head d-interleave -> [evens|odds]) so rope becomes half-block
    free-axis ops; q rows pre-scaled by D^-0.5; proj_w pre-transposed.
  - qkv = xT.T @ wT  (option A layout [n, o]) -> PSUM -> bf16 SBUF
  - rope applied as 6 DVE tensor_tensor ops per [128,1024] tile
  - q,k transposed per 128x128 block on the TensorEngine (identity mm)
    into qT/kT [d, n] layout for attention
  - S^T[j,i] = kT.T @ qT (K=64), exp on ScalarE (no max subtraction:
    |scores| <= ~6 for this input distribution).
  - fused-sums PV: v is stored interleaved per head as [v_h | ones64]
    ([128, 16, 128] SBUF tiles, ones lanes memset by GpSimd), so each
    PV matmul emits O^T rows 0..63 AND the softmax denominator
    replicated across rows 64..127 of the same [128, 256] PSUM tile —
    the row-sum selector matmuls and the reciprocal DRAM-bounce
    broadcast of the old design are gone. Normalization is local:
    reciprocal_approx_fast on rows 64..127 then one tensor_tensor mul
    into the bf16 oT tile.
  - y = O^T.T @ pwT + b, streamed out as fp32.
"""

import sys

if "/opt/trn_rl_repo" not in sys.path:
    sys.path.insert(0, "/opt/trn_rl_repo")

import numpy as np
import ml_dtypes

import concourse.bacc as bacc
import concourse.mybir as mybir
import concourse.tile as tile
from concourse.bass_utils import run_bass_kernel_spmd
from concourse.masks import make_identity

f32 = mybir.dt.float32
bf16 = mybir.dt.bfloat16

N_CORES = 8
B, N, C = 64, 256, 1024
H, D = 16, 64
B_LOC = B // N_CORES          # 8 batches per core
NT = B_LOC * N                # 2048 token rows per core
HW = 16
THETA = 10000.0

_cache = {}


def _rope_tables():
    hd = D // 2  # 32
    inv_freq = 1.0 / (THETA ** (np.arange(0, hd, 2, dtype=np.float32) / hd))
    t = np.arange(HW, dtype=np.float32)
    f = np.einsum("i,j->ij", t, inv_freq)          # (16, 16)
    f = np.repeat(f, 2, axis=-1)                   # (16, 32)
    fx = np.broadcast_to(f[:, None, :], (HW, HW, hd))
    fy = np.broadcast_to(f[None, :, :], (HW, HW, hd))
    F = np.concatenate([fx, fy], axis=-1).reshape(N, D)  # (256, 64)
    cosH = np.cos(F[:, 0::2])                      # (256, 32)
    sinH = np.sin(F[:, 0::2])
    return cosH.astype(np.float32), sinH.astype(np.float32)


def _build():
    if "nc" in _cache:
        return _cache["nc"]

    nc = bacc.Bacc("TRN2", target_bir_lowering=False, debug=False,
                   num_devices=N_CORES)

    xT_d = nc.dram_tensor("xT", [C, NT], bf16, kind="ExternalInput")
    w_d = nc.dram_tensor("wT", [C, 3 * C], bf16, kind="ExternalInput")
    pw_d = nc.dram_tensor("pwT", [C, C], bf16, kind="ExternalInput")
    pb_d = nc.dram_tensor("pb", [1, C], f32, kind="ExternalInput")
    cos_d = nc.dram_tensor("cosH", [N, 32], bf16, kind="ExternalInput")
    sin_d = nc.dram_tensor("sinH", [N, 32], bf16, kind="ExternalInput")
    out_d = nc.dram_tensor("out", [NT, C], f32, kind="ExternalOutput")

    Exp = mybir.ActivationFunctionType.Exp
    MUL = mybir.AluOpType.mult
    ADD = mybir.AluOpType.add
    SUB = mybir.AluOpType.subtract

    from contextlib import ExitStack
    with tile.TileContext(nc) as tc:
        with ExitStack() as ctx:
            const = ctx.enter_context(tc.tile_pool(name="const", bufs=1))
            xg_p = ctx.enter_context(tc.tile_pool(name="xg", bufs=2))
            qkraw_p = ctx.enter_context(tc.tile_pool(name="qkraw", bufs=2))
            tmp_p = ctx.enter_context(tc.tile_pool(name="tmp", bufs=4))
            qkrot_p = ctx.enter_context(tc.tile_pool(name="qkrot", bufs=9))
            v_p = ctx.enter_context(tc.tile_pool(name="vg", bufs=2))
            qkT_p = ctx.enter_context(tc.tile_pool(name="qkT", bufs=2))
            pT_p = ctx.enter_context(tc.tile_pool(name="pT", bufs=8))
            oT_p = ctx.enter_context(tc.tile_pool(name="oT", bufs=2))
            rsb_p = ctx.enter_context(tc.tile_pool(name="rsb", bufs=4))
            y_p = ctx.enter_context(tc.tile_pool(name="y", bufs=2))
            psmm_p = ctx.enter_context(tc.tile_pool(name="psmm", bufs=5, space="PSUM"))
            pso_p = ctx.enter_context(tc.tile_pool(name="pso", bufs=3, space="PSUM"))

            # ---- constants ----
            # split the 6MB weight load into per-chunk DMAs spread across
            # the per-engine DMA queues so they run in parallel and the
            # first qkv matmuls can start as soon as chunk 0 lands
            dma_engines = [nc.sync, nc.scalar, nc.gpsimd]
            w_sb = [[const.tile([128, C], bf16, name=f"w_sb{fp}_{cc}")
                     for cc in range(8)] for fp in range(3)]
            w_r = w_d.ap().rearrange("(co ci) (fp o) -> ci co fp o",
                                     ci=128, fp=3)
            cos_sb = const.tile([128, 2, 32], bf16)
            nc.sync.dma_start(cos_sb[:], cos_d.ap().rearrange(
                "(nt p) t -> p nt t", p=128))
            sin_sb = const.tile([128, 2, 32], bf16)
            nc.sync.dma_start(sin_sb[:], sin_d.ap().rearrange(
                "(nt p) t -> p nt t", p=128))
            ident = const.tile([128, 128], bf16)
            make_identity(nc, ident)
            # pw/pb are allocated here but loaded after group 0's qkv
            # emission (queue FIFO keeps them out of the critical startup
            # bandwidth window; they are first read ~60us in)
            pw_sb = [const.tile([128, C], bf16, name=f"pw_sb{cc}")
                     for cc in range(8)]
            pb_bc = const.tile([128, C], f32)
            pw_r = pw_d.ap().rearrange("(co ci) o -> ci co o", ci=128)

            xT_r = xT_d.ap().rearrange("(co ci) n -> ci co n", ci=128)
            pending = []

            for g in range(4):          # group = 2 batches (512 token cols)
                xg = [xg_p.tile([128, 512], bf16, tag=f"xg{cc}",
                                name=f"xg_{g}_{cc}") for cc in range(8)]
                if g == 0:
                    # startup-critical ordering: queues are FIFO, so land
                    # chunks in consumption order (xg chunk 0 first, then q
                    # weights (fp0), k (fp1), v (fp2)); the weight chunks
                    # are spread over four queues (sync/scalar/vector/
                    # tensor — the latter two are idle this early) so the
                    # first accumulation's 8 chunks arrive ~2x sooner.
                    # pw/pb wait until after the qkv emission.
                    nc.gpsimd.dma_start(xg[0][:], xT_r[:, 0, 0:512])
                    for cc in range(1, 8):
                        nc.gpsimd.dma_start(xg[cc][:], xT_r[:, cc, 0:512])
                    for fp in range(2):
                        for cc in range(8):
                            dma_engines[cc % 2].dma_start(
                                w_sb[fp][cc][:], w_r[:, cc, fp, :])
                    for cc in range(8):
                        nc.gpsimd.dma_start(
                            w_sb[2][cc][:], w_r[:, cc, 2, :])
                else:
                    for cc in range(8):
                        dma_engines[cc % 3].dma_start(
                            xg[cc][:], xT_r[:, cc, g * 512:(g + 1) * 512])

                # v is stored head-interleaved with ones lanes for the
                # fused-sums PV: slot h = [v_h | ones] (even h) or
                # [ones | v_h] (odd h) so O rows and sum rows alternate
                # PSUM halves per head parity and every normalize op is
                # 64-partition aligned.
                v_g = [v_p.tile([128, 16, 128], bf16, tag=f"vg{ns}",
                                name=f"v_{g}_{ns}") for ns in range(4)]
                for ns in range(4):
                    v4 = v_g[ns][:].rearrange("p (pr q) c -> p pr q c", q=2)
                    nc.gpsimd.memset(v4[:, :, 0, 64:128], 1.0)
                    nc.gpsimd.memset(v4[:, :, 1, 0:64], 1.0)
                qkT_g = [qkT_p.tile([128, 512], bf16, tag=f"qkT{fb}",
                                    name=f"qkT_{g}_{fb}") for fb in range(16)]
                rot_tiles = {}

                # ---- qkv matmuls + rope ----
                # the two 512-wide halves of each 1024-col output share the
                # same stationary lhsT per k-chunk, letting walrus reuse the
                # loaded weights between consecutive matmuls
                for fp in range(3):     # 0: q, 1: k, 2: v (1024 cols each)
                    for ns in range(4):
                        if fp < 2:
                            raw = qkraw_p.tile([128, H, D], bf16, tag="qkraw")
                            rawf = raw[:].rearrange("p h d -> p (h d)")
                        pss = [psmm_p.tile([128, 512], f32, tag="mm",
                                           name=f"ps_{fp}_{ns}_{i}")
                               for i in range(2)]
                        for cc in range(8):
                            for half in range(2):
                                fo = fp * 2 + half
                                nc.tensor.matmul(
                                    pss[half][:],
                                    lhsT=xg[cc][:, ns * 128:(ns + 1) * 128],
                                    rhs=w_sb[fp][cc][:, (fo % 2) * 512:(fo % 2 + 1) * 512],
                                    start=(cc == 0), stop=(cc == 7))
                        for half in range(2):
                            if fp == 2:
                                # heads half*8..half*8+7: even slots keep v
                                # in cols 0:64, odd slots in cols 64:128
                                v4 = v_g[ns][:].rearrange(
                                    "p (pr q) c -> p pr q c", q=2)
                                s4 = pss[half][:].rearrange(
                                    "p (pr q c) -> p pr q c", q=2, c=64)
                                nc.any.tensor_copy(
                                    out=v4[:, half * 4:(half + 1) * 4, 0, 0:64],
                                    in_=s4[:, :, 0, :])
                                nc.any.tensor_copy(
                                    out=v4[:, half * 4:(half + 1) * 4, 1, 64:128],
                                    in_=s4[:, :, 1, :])
                            else:
                                nc.any.tensor_copy(
                                    out=rawf[:, half * 512:(half + 1) * 512],
                                    in_=pss[half][:])
                        if fp == 2:
                            continue
                        # rope: evens = raw[:,:,0:32], odds = raw[:,:,32:64]
                        nt = ns % 2
                        cos = cos_sb[:, nt, None, :].to_broadcast((128, H, 32))
                        sin = sin_sb[:, nt, None, :].to_broadcast((128, H, 32))
                        qe = raw[:, :, 0:32]
                        qo = raw[:, :, 32:64]
                        t1 = tmp_p.tile([128, H, 32], bf16, tag="tmp")
                        t2 = tmp_p.tile([128, H, 32], bf16, tag="tmp")
                        nc.vector.tensor_tensor(out=t1[:], in0=qe, in1=cos, op=MUL)
                        nc.vector.tensor_tensor(out=t2[:], in0=qo, in1=sin, op=MUL)
                        rot = qkrot_p.tile([128, H, D], bf16, tag="rot")
                        nc.vector.tensor_tensor(out=rot[:, :, 0:32],
                                                in0=t1[:], in1=t2[:], op=SUB)
                        t3 = tmp_p.tile([128, H, 32], bf16, tag="tmp")
                        t4 = tmp_p.tile([128, H, 32], bf16, tag="tmp")
                        nc.vector.tensor_tensor(out=t3[:], in0=qo, in1=cos, op=MUL)
                        nc.vector.tensor_tensor(out=t4[:], in0=qe, in1=sin, op=MUL)
                        nc.vector.tensor_tensor(out=rot[:, :, 32:64],
                                                in0=t3[:], in1=t4[:], op=ADD)
                        rot_tiles[(fp, ns)] = rot

                # ---- transpose q,k into [d, n] layout (PE identity mm) ----
                for fb in range(16):    # 0..7 q blocks, 8..15 k blocks
                    fcol = (fb % 8) * 128
                    pst = psmm_p.tile([128, 512], bf16, tag="mm",
                                      name=f"pst_{g}_{fb}")
                    for ns in range(4):
                        rot = rot_tiles[(fb // 8, ns)]
                        nc.tensor.transpose(
                            pst[:, ns * 128:(ns + 1) * 128],
                            rot[:].rearrange("p h d -> p (h d)")[:, fcol:fcol + 128],
                            ident)
                    nc.any.tensor_copy(out=qkT_g[fb][:], in_=pst[:])
                if g == 0:
                    for cc in range(8):
                        dma_engines[cc % 3].dma_start(
                            pw_sb[cc][:], pw_r[:, cc, :])
                    nc.sync.dma_start(pb_bc[:],
                                      pb_d.ap().to_broadcast((128, C)))

                # ---- attention per batch ----
                # per-head pipeline: scores (PE) -> exp (ScalarE) ->
                # fused-sums PV (PE, O rows + replicated denominator in one
                # [128, 256] PSUM region) -> approx-reciprocal + normalize
                # mul (DVE, all local, no DMA). The previous batch's output
                # projection is spread through this batch's head loop in 4
                # chunks so its PSUM-accumulation tails overlap scores.
                for bb in range(2):
                    b_loc = 2 * g + bb
                    oT_b = oT_p.tile([128, 8, 256], bf16, tag="oT")
                    ps_pair = None
                    ssb = None
                    pTs = {}
                    # software-pipelined by one head: scores/exp of head
                    # `it` are emitted before PV of head `it-1`, so the
                    # in-order PE never stalls scores behind a PV that is
                    # still waiting on its exp.
                    for it in range(H + 1):
                        if it < H:
                            h = it
                            qfb, qrow = h // 2, (h % 2) * 64
                            kfb = 8 + h // 2
                            ps_s = psmm_p.tile([128, 2, 256], f32, tag="mm")
                            for jc in range(2):
                                nc.tensor.matmul(
                                    ps_s[:, jc, :],
                                    lhsT=qkT_g[kfb][qrow:qrow + 64,
                                                   bb * 256 + jc * 128:bb * 256 + jc * 128 + 128],
                                    rhs=qkT_g[qfb][qrow:qrow + 64,
                                                   bb * 256:bb * 256 + 256],
                                    start=True, stop=True)
                            pT = pT_p.tile([128, 2, 256], bf16, tag="pT")
                            nc.scalar.activation(pT[:], ps_s[:], Exp)
                            pTs[h] = pT
                        if it >= 1:
                            h = it - 1
                            pT = pTs.pop(h)
                            if h % 2 == 0:
                                ps_pair = pso_p.tile([128, 512], f32,
                                                     tag="o")
                            po = ps_pair[:,
                                         (h % 2) * 256:(h % 2) * 256 + 256]
                            for jc in range(2):
                                nc.tensor.matmul(
                                    po,
                                    lhsT=v_g[bb * 2 + jc][:].rearrange(
                                        "p s c -> p (s c)")[:, h * 128:(h + 1) * 128],
                                    rhs=pT[:, jc, :],
                                    start=(jc == 0), stop=(jc == 1))
                            # normalize per head pair: gather the two
                            # heads' replicated sums (aligned ScalarE
                            # copies; the custom-DVE approx reciprocal only
                            # works at base partition 0 full-window, and
                            # ScalarE handles the PSUM read), one
                            # reciprocal, two muls with cross-window in1
                            # reads (verified OK on hw).
                            if h % 2 == 0:
                                ssb = rsb_p.tile([128, 256], f32, tag="ssb")
                                nc.scalar.copy(out=ssb[64:128, :],
                                               in_=po[64:128, :])
                            else:
                                nc.scalar.copy(out=ssb[0:64, :],
                                               in_=po[0:64, :])
                                rsb = rsb_p.tile([128, 256], f32, tag="rsb")
                                nc.vector.reciprocal_approx_fast(rsb[:],
                                                                 ssb[:])
                                po_e = ps_pair[:, 0:256]
                                nc.vector.tensor_tensor(
                                    out=oT_b[0:64, h // 2, :],
                                    in0=po_e[0:64, :],
                                    in1=rsb[64:128, :], op=MUL)
                                nc.vector.tensor_tensor(
                                    out=oT_b[64:128, h // 2, :],
                                    in0=po[64:128, :], in1=rsb[0:64, :],
                                    op=MUL)
                        if it % 4 == 1 and pending:
                            pending.pop(0)()

                    # ---- output projection (deferred, 4 chunks) ----
                    def make_proj(b_loc, oT_b):
                        chunks = []
                        for nt2 in range(2):
                          for oc in range(2):
                            def chunk(nt2=nt2, oc=oc, b_loc=b_loc, oT_b=oT_b):
                                ps_p = psmm_p.tile([128, 2, 256], f32,
                                                   tag="mm")
                                ps_pv = ps_p[:].rearrange("p a b -> p (a b)")
                                for cc in range(8):
                                    nc.tensor.matmul(
                                        ps_pv,
                                        lhsT=oT_b[:, cc,
                                                  nt2 * 128:(nt2 + 1) * 128],
                                        rhs=pw_sb[cc][:,
                                                      oc * 512:(oc + 1) * 512],
                                        start=(cc == 0), stop=(cc == 7))
                                y_sb = y_p.tile([128, 512], f32, tag="y")
                                nc.vector.tensor_tensor(
                                    out=y_sb[:], in0=ps_pv,
                                    in1=pb_bc[:, oc * 512:(oc + 1) * 512],
                                    op=ADD)
                                row0 = b_loc * 256 + nt2 * 128
                                nc.sync.dma_start(
                                    out_d.ap()[row0:row0 + 128,
                                               oc * 512:(oc + 1) * 512],
                                    y_sb[:])
                            chunks.append(chunk)
                        return chunks
                    pending.extend(make_proj(b_loc, oT_b))

            while pending:
                pending.pop(0)()

    nc.compile()
    _cache["nc"] = nc
    return nc


def _prep_inputs(x, qkv_w, proj_w, proj_b):
    perm = np.concatenate([np.arange(0, D, 2), np.arange(1, D, 2)])  # evens|odds
    head_perm = (np.arange(H)[:, None] * D + perm[None, :]).reshape(-1)
    wq = qkv_w[:C][head_perm] * np.float32(D ** -0.5)
    wk = qkv_w[C:2 * C][head_perm]
    wv = qkv_w[2 * C:]
    wT = np.ascontiguousarray(
        np.concatenate([wq, wk, wv], 0).T).astype(ml_dtypes.bfloat16)
    pwT = np.ascontiguousarray(proj_w.T).astype(ml_dtypes.bfloat16)
    pb = np.ascontiguousarray(proj_b.reshape(1, C)).astype(np.float32)
    cosH, sinH = _rope_tables()
    cosH = cosH.astype(ml_dtypes.bfloat16)
    sinH = sinH.astype(ml_dtypes.bfloat16)

    in_maps = []
    for c in range(N_CORES):
        xs = x[c * B_LOC:(c + 1) * B_LOC].reshape(NT, C)
        xT = np.ascontiguousarray(xs.T).astype(ml_dtypes.bfloat16)
        in_maps.append({"xT": xT, "wT": wT, "pwT": pwT, "pb": pb,
                        "cosH": cosH, "sinH": sinH})
    return in_maps


def _run(inputs, trace=False, **kw):
    nc = _build()
    in_maps = _prep_inputs(inputs["x"], inputs["qkv_w"],
                           inputs["proj_w"], inputs["proj_b"])
    res = run_bass_kernel_spmd(nc, in_maps, core_ids=list(range(N_CORES)),
                               trace=trace, **kw)
    out = np.concatenate([res.results[c]["out"] for c in range(N_CORES)], 0)
    return out.reshape(B, N, C).astype(np.float32), res


def kernel(x, qkv_w, proj_w, proj_b):
    x = np.asarray(x, dtype=np.float32)
    qkv_w = np.asarray(qkv_w, dtype=np.float32)
    proj_w = np.asarray(proj_w, dtype=np.float32)
    proj_b = np.asarray(proj_b, dtype=np.float32)
    out, _ = _run({"x": x, "qkv_w": qkv_w, "proj_w": proj_w,
                   "proj_b": proj_b})
    return out

